# revision 5
# baseline (speedup 1.0000x reference)
"""Trainium2 Bass kernel for nn_DMCustom_28338194219111 (scatter_memory).

reference semantics: a DDPM pixel-swap degrade. A permutation of the
H*W=4096 pixels is built from (u1, u2, t) by sequentially composing
4096 transpositions; x[:, 0] is then gathered with that permutation.

Strategy (per the sharding hint): the permutation is batch-independent
and tiny -> computed on host (exact float32 replica of the jax math);
x is sharded over batch across 8 NeuronCores; each core performs its
local gather as DRAM->DRAM DMA copies whose access patterns bake in
the (host-computed) permutation, decomposed into maximal contiguous
runs. The bulk (identity) portion of the copy is split across both
HWDGE queue rings (sync/SP and scalar/ACT) and into multiple
dma_starts per ring: all 16 SDMA engines stream concurrently from two
rings, which measures ~1.5x faster than a single 16 MiB dma_start.
For the common t-regime (t <= ~780) the permutation is exactly the
identity and the kernel is the pure 16 MiB bulk copy per core.
"""

import numpy as np

H = W = 64
HW = H * W            # 4096
BATCH = 8192
N_CORES = 8
ROWS_PER_CORE = BATCH // N_CORES   # 1024
N_T = 1000
BETA1, BETA2 = 1e-4, 0.02

# Bulk-copy layout: (engine_name, [(row0, row1), ...]) per ring.
# Chosen by interleaved A/B measurement on the target cores.
BULK_LAYOUT = (
    ("sync", ((0, 256), (256, 512))),
    ("scalar", ((512, 768), (768, 1024))),
)

_nc_cache: dict = {}


def _compute_perm(u1: np.ndarray, u2: np.ndarray, t: int) -> np.ndarray:
    """Exact numpy replica of reference._swap_permutation (float32 ops)."""
    f32 = np.float32
    beta = f32(BETA2 - BETA1) * (f32(t) / f32(N_T)) + f32(BETA1)
    d1 = ((u1 - f32(0.5)) * f32(2.0) * beta * f32(H)).astype(np.int32)
    d2 = ((u2 - f32(0.5)) * f32(2.0) * beta * f32(W)).astype(np.int32)
    rows0, cols0 = np.meshgrid(np.arange(H, dtype=np.int32),
                               np.arange(W, dtype=np.int32), indexing="ij")
    tr = (rows0 + d2) % W
    tc = (cols0 + d1) % H
    q = (tr.astype(np.int64) * W + tc).reshape(-1)
    perm = np.arange(HW, dtype=np.int32)
    for i in range(HW):
        qi = q[i]
        vi = perm[i]
        perm[i] = perm[qi]
        perm[qi] = vi
    return perm


def _perm_runs(perm: np.ndarray) -> list[tuple[int, int, int]]:
    """Decompose perm into maximal runs (dst_start, src_start, length)
    with perm[dst_start + k] == src_start + k for k < length."""
    runs = []
    j = 0
    while j < HW:
        s = int(perm[j])
        L = 1
        while j + L < HW and int(perm[j + L]) == s + L:
            L += 1
        runs.append((j, s, L))
        j += L
    return runs


def _build_nc(perm: np.ndarray, reps: int = 1):
    """Per-core gather kernel: bulk identity copy split across both HWDGE
    rings (multiple dma_starts each), then permutation patches overwriting
    their destination columns, also split across the rings.

    reps>1 repeats the whole pass with a full cross-ring join between
    repetitions — used only for marginal-time measurement (fixed overheads
    cancel in the difference)."""
    import concourse.bass as bass
    import concourse.mybir as mybir

    runs = _perm_runs(perm)
    # non-identity segments only; the bulk copy covers the rest and the
    # patches overwrite their destination columns afterwards.
    patches = [(d, s, L) for d, s, L in runs if d != s]

    nc = bass.Bass()
    x = nc.declare_dram_parameter("x", [ROWS_PER_CORE, HW],
                                  mybir.dt.float32, isOutput=False)
    out = nc.declare_dram_parameter("out", [ROWS_PER_CORE, HW],
                                    mybir.dt.float32, isOutput=True)

    engines = [name for name, _ in BULK_LAYOUT]
    n_eng = len(engines)
    # per-engine per-rep semaphore increments: bulk chunks + patch share
    patch_share = [len(patches[i::n_eng]) for i in range(n_eng)]
    per_rep = [16 * (len(chunks) + patch_share[i])
               for i, (_, chunks) in enumerate(BULK_LAYOUT)]
    bulk_inc = [16 * len(chunks) for _, chunks in BULK_LAYOUT]

    with (
        nc.Block() as block,
        nc.semaphore("s0") as s0,
        nc.semaphore("s1") as s1,
    ):
        sems = [s0, s1][:n_eng]

        def make_emit(idx, chunks):
            sem = sems[idx]
            my_patches = patches[idx::n_eng]

            def emit(eng):
                for rep in range(reps):
                    # join: all engines' previous rep fully done (WAW)
                    if rep:
                        for j in range(n_eng):
                            eng.wait_ge(sems[j], per_rep[j] * rep)
                    base = per_rep[idx] * rep
                    for (r0, r1) in chunks:
                        eng.dma_start(out=out[r0:r1, :],
                                      in_=x[r0:r1, :]).then_inc(sem, 16)
                    if my_patches:
                        # patches overwrite destination columns after the
                        # bulk copy of every row range (WAW) — wait for all
                        # engines' bulk of THIS rep.
                        for j in range(n_eng):
                            eng.wait_ge(sems[j], per_rep[j] * rep + bulk_inc[j])
                        with nc.allow_non_contiguous_dma(
                                reason="per-pixel permutation patches"):
                            for (dst, src, L) in my_patches:
                                eng.dma_start(
                                    out=out[:, dst:dst + L],
                                    in_=x[:, src:src + L]).then_inc(sem, 16)
                    eng.wait_ge(sem, base + per_rep[idx])
            return emit

        for idx, (name, chunks) in enumerate(BULK_LAYOUT):
            getattr(block, name)(make_emit(idx, chunks))

    return nc


def _make_sharded_fn(nc, donate: bool = False):
    """Mirror bass2jax.run_bass_via_pjrt's multi-core path (including the
    trailing partition_id operand the NEFF expects). donate=False lets
    device-resident inputs be reused across timed calls."""
    import jax
    from jax.sharding import Mesh, PartitionSpec, NamedSharding
    from jax.experimental.shard_map import shard_map
    from concourse import bass2jax

    bass2jax.install_neuronx_cc_hook()
    out_avals = [jax.core.ShapedArray((ROWS_PER_CORE, HW), np.float32)]
    pname = nc.partition_id_tensor.name if nc.partition_id_tensor else None
    in_names = ["x", "out"] + ([pname] if pname else [])

    def _body(*args):
        operands = list(args)
        if pname:
            operands.append(bass2jax.partition_id_tensor())
        outs = bass2jax._bass_exec_p.bind(
            *operands,
            out_avals=tuple(out_avals),
            in_names=tuple(in_names),
            out_names=("out",),
            lowering_input_output_aliases=(),
            sim_require_finite=True,
            sim_require_nnan=True,
            nc=nc,
        )
        return tuple(outs)

    devices = jax.devices()[:N_CORES]
    mesh = Mesh(np.asarray(devices), ("core",))
    fn = jax.jit(
        shard_map(
            _body, mesh=mesh,
            in_specs=(PartitionSpec("core"),) * 2,
            out_specs=(PartitionSpec("core"),),
            check_rep=False,
        ),
        **({"donate_argnums": (1,)} if donate else {}),
        keep_unused=True,
    )
    sharding = NamedSharding(mesh, PartitionSpec("core"))
    return fn, sharding


def time_device_exec(inputs, reps: int = 513, kcalls: int = 6,
                     rounds: int = 10) -> int:
    """Measure the marginal device time of one full gather pass.

    Host-side dispatch over the axon tunnel costs ~73 ms per execution
    with ms-scale jitter, so a single (T[reps]-T[1]) pair is far too
    noisy. Each timing point queues `kcalls` executions of the reps-copy
    NEFF asynchronously and blocks once, amortizing the jitter over
    kcalls*(reps-1) copies:
        marginal = (T[k x reps] - T[k x 1]) / (k*(reps-1))
    With reps=513 the pairing noise is ~5 us; the median over rounds is
    reported (min would be biased low by pairing-noise tails)."""
    import jax, time

    x = np.asarray(inputs["x"], dtype=np.float32)
    u1 = np.asarray(inputs["u1"], dtype=np.float32)
    u2 = np.asarray(inputs["u2"], dtype=np.float32)
    t = int(np.asarray(inputs["t"]))
    perm = _compute_perm(u1, u2, t)

    xf = np.ascontiguousarray(x.reshape(BATCH, HW))
    zeros = np.zeros_like(xf)

    fns = {}
    for r in (1, reps):
        nc = _build_nc(perm, reps=r)
        fn, sharding = _make_sharded_fn(nc)
        dx = jax.device_put(xf, sharding)
        dz = jax.device_put(zeros, sharding)
        fn(dx, dz)[0].block_until_ready()          # warmup/compile
        fns[r] = (fn, dx, dz)

    def timed_batch(r):
        fn, dx, dz = fns[r]
        t0 = time.perf_counter()
        outs = [fn(dx, dz)[0] for _ in range(kcalls)]
        for o in outs:
            o.block_until_ready()
        return time.perf_counter() - t0

    marginals = []
    for _ in range(rounds):
        t1 = timed_batch(1)
        tr = timed_batch(reps)
        marginals.append((tr - t1) / (kcalls * (reps - 1)))
    lo = min(marginals)
    med = float(np.median(marginals))
    print(f"  marginal/copy: median {med * 1e6:.1f} us, min {lo * 1e6:.1f} us "
          f"over {rounds} batched rounds ({kcalls}x{reps} copies each)")
    return max(0, int(med * 1e9))


def _get_exec(perm: np.ndarray):
    """Cached (jitted_fn, zeros_maker, sharding) for this permutation."""
    key = perm.tobytes()
    entry = _nc_cache.get(key)
    if entry is None:
        import jax
        import jax.numpy as jnp

        nc = _build_nc(perm)
        fn, sharding = _make_sharded_fn(nc, donate=True)
        # "out" is fully overwritten (perm is a bijection), so its initial
        # contents are irrelevant — make the donated buffer on device
        # instead of uploading 128 MiB of zeros.
        zeros_maker = jax.jit(
            lambda: jnp.zeros((BATCH, HW), jnp.float32),
            out_shardings=sharding,
        )
        entry = (fn, zeros_maker, sharding)
        _nc_cache[key] = entry
    return entry


def kernel(x, u1, u2, t):
    import jax

    x = np.asarray(x, dtype=np.float32)
    u1 = np.asarray(u1, dtype=np.float32)
    u2 = np.asarray(u2, dtype=np.float32)
    t = int(np.asarray(t))

    perm = _compute_perm(u1, u2, t)
    fn, zeros_maker, sharding = _get_exec(perm)

    xf = np.ascontiguousarray(x.reshape(BATCH, HW))
    dx = jax.device_put(xf, sharding)
    out = fn(dx, zeros_maker())[0]
    return np.asarray(out).reshape(BATCH, 1, H, W)
